# revision 31
# baseline (speedup 1.0000x reference)
"""Trainium2 Bass kernel for local (Gaussian-windowed) attention.

Reference computation (per batch b):
    h = target[b]                                # [D]
    p = sigmoid(tanh(h @ Wp + bp) @ Vp + bv) * S # scalar aligned position
    a = h @ Wa + ba                              # [D]
    x[s, d]  = source[b, s, d] * a[d]
    y[s, :]  = softmax(x[s, :])                  # over feature axis
    w[s, :]  = softmax(y[s, :])                  # double softmax
    g[s]     = exp(-2 * ((s - p) / 50)^2)        # Gaussian window
    out[b,d] = sum_s w[s, d] * g[s] * src[b, s, d]

Gaussian window width 50 -> only a 128-position window of `source` around p
matters.  The window offset is computed on-device and used as a
register-dynamic DMA offset.

Key structure:
  - Row-packed p-chain: lhsT = [tgt_hi | tgt_lo] (8 columns) against
    rhs = Wp_hi_k gives hi@hi and lo@hi in ONE matmul (PE cost depends
    only on the free dim), and against Wp_lo_k gives hi@lo.  9 matmuls
    total instead of 13; the row-block sums run on DVE.
  - Two-phase p: tanh of the hi@hi rows (read mid-accumulation) gives an
    approximate p good to ~+-4 positions -- enough to *center* the
    128-position window, so the window DMAs launch ~6us before the exact
    p (needed only for the Gaussian factors g) is finished.
  - DMA rings: FIFO per HWDGE ring gives priority: sync ring carries
    Wp-hi chunks, Wp-lo, Wa; scalar ring the small constant packs, then
    windows alternate rings.
  - Stream per batch: x = win*a (DVE) -> e1 = exp(x) bf16 + accum s1
    (ACT) -> r1 = 1/s1 (DVE) -> e2 = exp(e1*r1) bf16 + accum s2 (ACT
    scale port) -> t3 = e2*win bf16 (DVE) -> matmul with lhsT = one-hot
    column of wv = g/s2 in bf16 (Gaussian factor and second-softmax
    normalisation ride the PE lhsT, off the t3 path).
"""

import os
from contextlib import ExitStack

import numpy as np

import concourse.bass as bass
import concourse.tile as tile
from concourse import bacc, mybir
from concourse.bass_utils import run_bass_kernel_spmd

F32 = mybir.dt.float32
BF16 = mybir.dt.bfloat16
I32 = mybir.dt.int32
AF = mybir.ActivationFunctionType
OP = mybir.AluOpType
ET = mybir.EngineType

N_CORES = 8
B, S, D = 32, 4096, 512
BPC = B // N_CORES          # batches per core
KP = D // 128               # contraction chunks of 128 for D=512
WINDOW = 50.0

WT = 1                      # window tiles; window = 128*WT positions
WPOS = 128 * WT
S0_MAX = S - WPOS
PWH_W = D + 32 + BPC        # packWh cols: Wp_hi | tgt_hi | pad | tgt_lo@32
PA_W = KP * D + KP * BPC    # packA cols: Wa chunks | tgt chunks
PT_W = D + 2                # ptail cols: vp4 | bvh | pad
CST_W = WT + BPC            # cst cols: io50 | ident4
BR_W = 2 * D + BPC * 128    # brow cols: bp | ba | sel rows

DBG_STAGE = int(os.environ.get("DBG_STAGE", "4"))


def _emit(ctx: ExitStack, tc: tile.TileContext, outs, ins):
    nc = tc.nc
    (out,) = outs
    (src, packWh, packWl, packA, ptail, brow, cst) = ins

    sb = ctx.enter_context(tc.tile_pool(name="sb", bufs=1))
    ps = ctx.enter_context(tc.tile_pool(name="ps", bufs=1, space="PSUM"))
    psk = ctx.enter_context(tc.tile_pool(name="psk", bufs=1, space="PSUM"))

    def const(name, shape, dtype=F32):
        return sb.tile(shape, dtype, tag=name, name=name)

    # ---- memset constants --------------------------------------------------
    ones1_bf = const("ones1_bf", [1, BPC], BF16)
    nc.gpsimd.memset(ones1_bf[:], 1.0)
    tr_in = const("tr_in", [32, 32])
    nc.gpsimd.memset(tr_in[:], 0.0)
    tr_out = const("tr_out", [32, 32])

    # ---- weight DMAs (ring FIFO gives priority ordering) ------------------
    # sync ring: Wp-hi chunk A, Wp-hi chunk B, Wp-lo, Wa pack
    # scalar ring: small constant packs (ptail, cst, brow)
    pwh = const("pwh", [128, KP, PWH_W], BF16)
    nc.sync.dma_start(pwh[:, 0:2, :], packWh[:, 0:2, :])
    nc.sync.dma_start(pwh[:, 2:4, :], packWh[:, 2:4, :])
    pwl = const("pwl", [128, KP, D], BF16)
    nc.sync.dma_start(pwl[:], packWl[:])
    pa = const("pa", [128, PA_W], BF16)
    nc.sync.dma_start(pa[:], packA[:])
    pt = const("pt", [BPC, PT_W])
    nc.scalar.dma_start(pt[:], ptail[:])
    cs = const("cs", [128, CST_W])
    nc.scalar.dma_start(cs[:], cst[:])
    br = const("br", [BPC, BR_W], BF16)
    nc.scalar.dma_start(br[:], brow[:])

    vp4 = pt[:, 0:D]
    bvh_ap = pt[0:1, D : D + 1]
    io50 = cs[:, 0:WT]
    ident4 = cs[0:BPC, WT : WT + BPC]

    # zeroed one-hot wv holders + misc (gpsimd is otherwise idle early)
    ghb = {}
    for b in range(BPC):
        ghb[b] = const(f"ghb{b}", [128, BPC], BF16)
        nc.gpsimd.memset(ghb[b][:], 0.0)
    ones_f = const("ones_f", [1, 128])
    nc.gpsimd.memset(ones_f[:], 1.0)

    if DBG_STAGE <= 0:
        out_sb0 = const("out_sb0", [BPC, D])
        nc.vector.tensor_copy(out_sb0[:], pwh[0:BPC, 0, 0:D])
        nc.sync.dma_start(out[:], out_sb0[:])
        return

    # ---- p-chain: P1 = [tgt_hi|tgt_lo]^T @ Wp_hi  (8 rows) ----------------
    psum_p1 = ps.tile([32 + BPC, D], F32, tag="u", name="psum_p1")
    for k in range(KP):
        nc.tensor.matmul(psum_p1[:],
                         lhsT=pwh[:, k, D : D + 32 + BPC],
                         rhs=pwh[:, k, 0:D],
                         start=(k == 0), stop=False)

    with tc.high_priority():
        th = const("th", [BPC, D])
        nc.scalar.activation(th[:], psum_p1[0:BPC, :], AF.Tanh)
        ttr_junk = const("ttr_junk", [BPC, D])
        nc.vector.scalar_tensor_tensor(ttr_junk[:], th[:], 1.0, vp4,
                                       op0=OP.mult, op1=OP.mult,
                                       accum_out=tr_in[0:BPC, 0:1])
        nc.vector.transpose(tr_out[:], tr_in[:])
        # sigmoid(v+bv) = 0.5*tanh(0.5*v + 0.5*bv) + 0.5
        th2_a = const("th2_a", [1, BPC])
        nc.scalar.activation(th2_a[:], tr_out[0:1, 0:BPC], AF.Tanh,
                             bias=bvh_ap, scale=0.5)

        if DBG_STAGE <= 1:
            out_sb1 = const("out_sb1", [BPC, D])
            nc.vector.tensor_copy(out_sb1[:], th[:])
            nc.sync.dma_start(out[:], out_sb1[:])
            return

        # window start: s0 = clamp(trunc(p_approx) - WPOS/2, 0, S - WPOS)
        cf_row = const("cf_row", [1, BPC])
        nc.gpsimd.tensor_scalar(cf_row[:], th2_a[:], float(S) / 2.0,
                                float(S) / 2.0 - WPOS / 2.0,
                                op0=OP.mult, op1=OP.add)
        t0i_row = const("t0i_row", [1, BPC], I32)
        nc.gpsimd.tensor_scalar(t0i_row[:], cf_row[:], 0.0, float(S0_MAX),
                                op0=OP.max, op1=OP.min)
        _, t0v = nc.values_load_multi_w_load_instructions(
            t0i_row[:1, 0:BPC], engines=(ET.SP, ET.Activation),
            min_val=0, max_val=S0_MAX, skip_runtime_bounds_check=True)

        win = {}
        for b in range(BPC):
            win[b] = sb.tile([128, WT, D], F32, tag=f"win{b}", name=f"win{b}")
            eng = nc.sync if b % 2 == 0 else nc.scalar
            eng.dma_start(
                win[b][:],
                src[b][bass.ds(t0v[b], WPOS), :].rearrange("(t p) d -> p t d",
                                                           p=128))

    if DBG_STAGE <= 2:
        out_sb2 = const("out_sb2", [BPC, D])
        for b in range(BPC):
            nc.vector.tensor_copy(out_sb2[b : b + 1, :], win[b][0:1, 0, :])
        nc.sync.dma_start(out[:], out_sb2[:])
        return

    # the lo@hi rows are final after the A-matmuls; stash them in SBUF
    # (off the critical path) so the exact-u add has only one PSUM operand
    lo_sb = const("lo_sb", [BPC, D])
    nc.vector.tensor_copy(lo_sb[:], psum_p1[32 : 32 + BPC, :])
    # bp + hi@lo accumulate into the hi@hi rows after the approx tanh read
    nc.tensor.matmul(psum_p1[0:BPC, :], lhsT=ones1_bf[:], rhs=br[0:1, 0:D],
                     start=False, stop=False)
    for k in range(KP):
        nc.tensor.matmul(psum_p1[0:BPC, :],
                         lhsT=pwh[:, k, D : D + BPC],
                         rhs=pwl[:, k, :],
                         start=False, stop=(k == KP - 1))

    # ---- a-chain matmuls --------------------------------------------------
    psum_a = ps.tile([BPC, D], F32, tag="a", name="psum_a")
    for k in range(KP):
        nc.tensor.matmul(psum_a[:],
                         lhsT=pa[:, KP * D + BPC * k : KP * D + BPC * (k + 1)],
                         rhs=pa[:, k * D : (k + 1) * D],
                         start=(k == 0), stop=False)
    nc.tensor.matmul(psum_a[:], lhsT=ones1_bf[:], rhs=br[0:1, D : 2 * D],
                     start=False, stop=True)
    a_bf = const("a_bf", [BPC, D], BF16)
    nc.vector.tensor_copy(a_bf[:], psum_a[:])
    ab = ps.tile([128, BPC, D], F32, tag="ab", name="psum_ab")
    for b in range(BPC):
        sel_b = br[:, 2 * D + b * 128 : 2 * D + (b + 1) * 128]
        nc.tensor.matmul(ab[:, b, :], lhsT=sel_b, rhs=a_bf[:],
                         start=True, stop=True)

    # ---- exact u = hi@hi+bp + lo@hi + hi@lo  ->  exact p for g ------------
    if True:
        u_e = const("u_e", [BPC, D])
        nc.vector.scalar_tensor_tensor(u_e[:], psum_p1[0:BPC, :], 1.0,
                                       lo_sb[:], op0=OP.mult, op1=OP.add)
        th_e = const("th_e", [BPC, D])
        nc.scalar.activation(th_e[:], u_e[:], AF.Tanh)
        nc.vector.scalar_tensor_tensor(ttr_junk[:], th_e[:], 1.0, vp4,
                                       op0=OP.mult, op1=OP.mult,
                                       accum_out=tr_in[0:BPC, 0:1])
        nc.vector.transpose(tr_out[:], tr_in[:])
        th2_e = const("th2_e", [1, BPC])
        nc.scalar.activation(th2_e[:], tr_out[0:1, 0:BPC], AF.Tanh,
                             bias=bvh_ap, scale=0.5)

        # g chain: q = t0/50 - p_exact/50, g = exp(-2*(io/50 + q)^2)
        t0f_row = const("t0f_row", [1, BPC])
        nc.gpsimd.tensor_copy(t0f_row[:], t0i_row[:])
        p50_row = const("p50_row", [1, BPC])
        nc.gpsimd.tensor_scalar(p50_row[:], th2_e[:], float(S) / WINDOW / 2.0,
                                float(S) / WINDOW / 2.0,
                                op0=OP.mult, op1=OP.add)
        q_row = const("q_row", [1, BPC])
        nc.gpsimd.tensor_scalar_mul(q_row[:], t0f_row[:], 1.0 / WINDOW)
        nc.gpsimd.tensor_tensor(q_row[:], q_row[:], p50_row[:],
                                op=OP.subtract)
        psum_q = ps.tile([128, BPC], F32, tag="sr", name="psum_q")
        nc.tensor.matmul(psum_q[:], lhsT=ones_f[:], rhs=q_row[:],
                         start=True, stop=True)
        q_bc = const("q_bc", [128, BPC])
        nc.vector.tensor_copy(q_bc[:], psum_q[:])
        ut4 = const("ut4", [128, BPC, WT])
        for b in range(BPC):
            nc.gpsimd.tensor_scalar_add(ut4[:, b, :], io50,
                                        q_bc[:, b : b + 1])
        ut4f = ut4[:].rearrange("p a b -> p (a b)")
        sq4 = const("sq4", [128, BPC * WT])
        nc.gpsimd.tensor_tensor(sq4[:], ut4f, ut4f, op=OP.mult)
        g4 = const("g4", [128, BPC, WT])
        nc.scalar.activation(g4[:].rearrange("p a b -> p (a b)"), sq4[:],
                             AF.Exp, scale=-2.0)

    # ---- per-batch softmax stream -----------------------------------------
    s1c = const("s1c", [128, BPC])
    r1c = const("r1c", [128, BPC])
    s2c = const("s2c", [128, BPC])
    r2c = const("r2c", [128, BPC])
    psum_ctx = psk.tile([BPC, D], F32, tag="ctx", name="psum_ctx")
    for b in range(BPC):
        wb = win[b]
        x = sb.tile([128, WT, D], F32, tag=f"x{b % 2}", name=f"x{b}")
        for t in range(WT):
            nc.vector.tensor_tensor(x[:, t, :], wb[:, t, :], ab[:, b, :],
                                    op=OP.mult)
        e1 = sb.tile([128, WT, D], BF16, tag=f"e1_{b % 2}", name=f"e1_{b}")
        for t in range(WT):
            nc.scalar.activation(e1[:, t, :], x[:, t, :], AF.Exp,
                                 accum_out=s1c[:, b : b + 1])
        nc.vector.reciprocal(r1c[:, b : b + 1], s1c[:, b : b + 1])
        e2 = sb.tile([128, WT, D], BF16, tag=f"e2_{b % 2}", name=f"e2_{b}")
        for t in range(WT):
            nc.scalar.activation(e2[:, t, :], e1[:, t, :], AF.Exp,
                                 scale=r1c[:, b : b + 1],
                                 accum_out=s2c[:, b : b + 1])
        nc.vector.reciprocal(r2c[:, b : b + 1], s2c[:, b : b + 1])
        nc.vector.tensor_tensor(ghb[b][:, b : b + 1], g4[:, b, 0:1],
                                r2c[:, b : b + 1], op=OP.mult)
        t3 = sb.tile([128, WT, D], BF16, tag=f"t3_{b % 2}", name=f"t3_{b}")
        for t in range(WT):
            nc.vector.tensor_tensor(t3[:, t, :], e2[:, t, :], wb[:, t, :],
                                    op=OP.mult)
        for t in range(WT):
            nc.tensor.matmul(psum_ctx[:], lhsT=ghb[b][:], rhs=t3[:, t, :],
                             start=(b == 0 and t == 0),
                             stop=(b == BPC - 1 and t == WT - 1))

    out_sb = const("out_sb", [BPC, D])
    nc.vector.tensor_copy(out_sb[:], psum_ctx[:])
    nc.sync.dma_start(out[:], out_sb[:])


def build_nc():
    nc = bacc.Bacc("TRN2", target_bir_lowering=False, debug=False,
                   num_devices=N_CORES)
    src = nc.dram_tensor("source", [BPC, S, D], F32, kind="ExternalInput").ap()
    packWh = nc.dram_tensor("packWh", [128, KP, PWH_W], BF16,
                            kind="ExternalInput").ap()
    packWl = nc.dram_tensor("packWl", [128, KP, D], BF16,
                            kind="ExternalInput").ap()
    packA = nc.dram_tensor("packA", [128, PA_W], BF16,
                           kind="ExternalInput").ap()
    ptail = nc.dram_tensor("ptail", [BPC, PT_W], F32,
                           kind="ExternalInput").ap()
    brow = nc.dram_tensor("brow", [BPC, BR_W], BF16,
                          kind="ExternalInput").ap()
    cst = nc.dram_tensor("cst", [128, CST_W], F32, kind="ExternalInput").ap()
    out = nc.dram_tensor("out", [BPC, D], F32, kind="ExternalOutput").ap()
    with tile.TileContext(nc) as tc:
        with ExitStack() as ctx:
            _emit(ctx, tc, [out],
                  [src, packWh, packWl, packA, ptail, brow, cst])
    nc.compile()
    return nc


_NC_CACHE = {}


def _get_nc():
    if "nc" not in _NC_CACHE:
        _NC_CACHE["nc"] = build_nc()
    return _NC_CACHE["nc"]


def pack_weights(target_shard, Wp, bp, Wa, ba, Vp, bv):
    """Build the packed weight arrays for one core."""
    import ml_dtypes
    f = np.float32
    wp32 = np.asarray(Wp, f)
    wp_hi = wp32.astype(ml_dtypes.bfloat16)
    wp_lo = (wp32 - wp_hi.astype(f)).astype(ml_dtypes.bfloat16)
    tgt32 = np.asarray(target_shard, f).T                   # [D, BPC]
    tgt_hi = tgt32.astype(ml_dtypes.bfloat16)
    tgt_lo = (tgt32 - tgt_hi.astype(f)).astype(ml_dtypes.bfloat16)
    # per k-chunk: [128, D] Wp_hi rows | [128, BPC] tgt_hi | [128, BPC] tgt_lo
    wph_r = wp_hi.reshape(KP, 128, D)
    tgh_r = tgt_hi.reshape(KP, 128, BPC)
    tgl_r = tgt_lo.reshape(KP, 128, BPC)
    pad_r = np.zeros((KP, 128, 32 - BPC), ml_dtypes.bfloat16)
    packWh = (np.concatenate([wph_r, tgh_r, pad_r, tgl_r], axis=2)
              .transpose(1, 0, 2))                          # [128, KP, D+36]
    packWl = wp_lo.reshape(KP, 128, D).transpose(1, 0, 2)   # [128, KP, D]
    wa_bf = (np.asarray(Wa, f).reshape(KP, 128, D).transpose(1, 0, 2)
             .reshape(128, KP * D).astype(ml_dtypes.bfloat16))
    tgt_bf = (tgt_hi.reshape(KP, 128, BPC).transpose(1, 0, 2)
              .reshape(128, KP * BPC))
    packA = np.concatenate([wa_bf, tgt_bf], axis=1)         # [128, PA_W]
    vp4 = np.broadcast_to(np.asarray(Vp, f).ravel()[None, :], (BPC, D))
    tailc = np.zeros((BPC, 2), f)
    tailc[0, 0] = 0.5 * np.asarray(bv, f).ravel()[0]
    ptail = np.concatenate([vp4, tailc], axis=1)            # [BPC, D+2]
    brow = np.zeros((BPC, BR_W), ml_dtypes.bfloat16)
    brow[0, :D] = np.asarray(bp, f).ravel().astype(ml_dtypes.bfloat16)
    brow[0, D : 2 * D] = np.asarray(ba, f).ravel().astype(ml_dtypes.bfloat16)
    for b in range(BPC):
        brow[b, 2 * D + b * 128 : 2 * D + (b + 1) * 128] = 1.0
    cst = np.zeros((128, CST_W), f)
    io = np.arange(128, dtype=f)[:, None] + 128.0 * np.arange(WT, dtype=f)
    cst[:, 0:WT] = io / np.float32(WINDOW)
    cst[0:BPC, WT : WT + BPC] = np.eye(BPC, dtype=f)
    return (np.ascontiguousarray(packWh), np.ascontiguousarray(packWl),
            np.ascontiguousarray(packA), np.ascontiguousarray(ptail),
            np.ascontiguousarray(brow), np.ascontiguousarray(cst))


def make_in_maps(source, target, Wp, bp, Wa, ba, Vp, bv):
    in_maps = []
    for c in range(N_CORES):
        bs = slice(c * BPC, (c + 1) * BPC)
        packWh, packWl, packA, ptail, brow, cst = pack_weights(
            target[bs], Wp, bp, Wa, ba, Vp, bv)
        in_maps.append({
            "source": np.ascontiguousarray(source[bs], dtype=np.float32),
            "packWh": packWh, "packWl": packWl, "packA": packA,
            "ptail": ptail, "brow": brow, "cst": cst,
        })
    return in_maps


def kernel(source, target, Wp, bp, Wa, ba, Vp, bv, **run_kwargs):
    nc = _get_nc()
    in_maps = make_in_maps(source, target, Wp, bp, Wa, ba, Vp, bv)
    res = run_bass_kernel_spmd(nc, in_maps, core_ids=list(range(N_CORES)),
                               **run_kwargs)
    out = np.concatenate([r["out"] for r in res.results], axis=0)
    kernel.last_results = res
    return out


# revision 32
# speedup vs baseline: 1.0059x; 1.0059x over previous
"""Trainium2 Bass kernel for local (Gaussian-windowed) attention.

Reference computation (per batch b):
    h = target[b]                                # [D]
    p = sigmoid(tanh(h @ Wp + bp) @ Vp + bv) * S # scalar aligned position
    a = h @ Wa + ba                              # [D]
    x[s, d]  = source[b, s, d] * a[d]
    y[s, :]  = softmax(x[s, :])                  # over feature axis
    w[s, :]  = softmax(y[s, :])                  # double softmax
    g[s]     = exp(-2 * ((s - p) / 50)^2)        # Gaussian window
    out[b,d] = sum_s w[s, d] * g[s] * src[b, s, d]

Gaussian window width 50 -> only a 128-position window of `source` around p
matters.  The window offset is computed on-device and used as a
register-dynamic DMA offset.

Key structure:
  - Row-packed p-chain: lhsT = [tgt_hi | tgt_lo] (8 columns) against
    rhs = Wp_hi_k gives hi@hi and lo@hi in ONE matmul (PE cost depends
    only on the free dim), and against Wp_lo_k gives hi@lo.  9 matmuls
    total instead of 13; the row-block sums run on DVE.
  - Two-phase p: tanh of the hi@hi rows (read mid-accumulation) gives an
    approximate p good to ~+-4 positions -- enough to *center* the
    128-position window, so the window DMAs launch ~6us before the exact
    p (needed only for the Gaussian factors g) is finished.
  - DMA rings: FIFO per HWDGE ring gives priority: sync ring carries
    Wp-hi chunks, Wp-lo, Wa; scalar ring the small constant packs, then
    windows alternate rings.
  - Stream per batch: x = win*a (DVE) -> e1 = exp(x) bf16 + accum s1
    (ACT) -> r1 = 1/s1 (DVE) -> e2 = exp(e1*r1) bf16 + accum s2 (ACT
    scale port) -> t3 = e2*win bf16 (DVE) -> matmul with lhsT = one-hot
    column of wv = g/s2 in bf16 (Gaussian factor and second-softmax
    normalisation ride the PE lhsT, off the t3 path).
"""

import os
from contextlib import ExitStack

import numpy as np

import concourse.bass as bass
import concourse.tile as tile
from concourse import bacc, mybir
from concourse.bass_utils import run_bass_kernel_spmd

F32 = mybir.dt.float32
BF16 = mybir.dt.bfloat16
I32 = mybir.dt.int32
AF = mybir.ActivationFunctionType
OP = mybir.AluOpType
ET = mybir.EngineType

N_CORES = 8
B, S, D = 32, 4096, 512
BPC = B // N_CORES          # batches per core
KP = D // 128               # contraction chunks of 128 for D=512
WINDOW = 50.0

WT = 1                      # window tiles; window = 128*WT positions
WPOS = 128 * WT
S0_MAX = S - WPOS
PWH_W = D + 32 + BPC        # packWh cols: Wp_hi | tgt_hi | pad | tgt_lo@32
PA_W = KP * D + KP * BPC    # packA cols: Wa chunks | tgt chunks
PT_W = 2 * D + 2            # ptail cols: vp4 | bvh | pad | bp4
CST_W = WT + BPC            # cst cols: io50 | ident4
BR_W = BPC * 128 + D        # brow cols: sel rows | ba4

DBG_STAGE = int(os.environ.get("DBG_STAGE", "4"))


def _emit(ctx: ExitStack, tc: tile.TileContext, outs, ins):
    nc = tc.nc
    (out,) = outs
    (src, packWh, packWl, packA, ptail, brow, cst) = ins

    sb = ctx.enter_context(tc.tile_pool(name="sb", bufs=1))
    ps = ctx.enter_context(tc.tile_pool(name="ps", bufs=1, space="PSUM"))
    psk = ctx.enter_context(tc.tile_pool(name="psk", bufs=1, space="PSUM"))

    def const(name, shape, dtype=F32):
        return sb.tile(shape, dtype, tag=name, name=name)

    # ---- memset constants --------------------------------------------------
    tr_in = const("tr_in", [32, 32])
    nc.gpsimd.memset(tr_in[:], 0.0)
    tr_out = const("tr_out", [32, 32])

    # ---- weight DMAs (ring FIFO gives priority ordering) ------------------
    # sync ring: Wp-hi chunk A, Wp-hi chunk B, Wp-lo, Wa pack
    # scalar ring: small constant packs (ptail, cst, brow)
    pwh = const("pwh", [128, KP, PWH_W], BF16)
    nc.sync.dma_start(pwh[:, 0:2, :], packWh[:, 0:2, :])
    nc.sync.dma_start(pwh[:, 2:4, :], packWh[:, 2:4, :])
    pwl = const("pwl", [128, KP, D], BF16)
    nc.sync.dma_start(pwl[:], packWl[:])
    pa = const("pa", [128, PA_W], BF16)
    nc.sync.dma_start(pa[:], packA[:])
    pt = const("pt", [BPC, PT_W])
    nc.scalar.dma_start(pt[:], ptail[:])
    cs = const("cs", [128, CST_W])
    nc.scalar.dma_start(cs[:], cst[:])
    br = const("br", [BPC, BR_W], BF16)
    nc.scalar.dma_start(br[:], brow[:])

    vp4 = pt[:, 0:D]
    bvh_ap = pt[0:1, D : D + 1]
    bp4 = pt[:, D + 2 : 2 * D + 2]
    ba4 = br[:, BPC * 128 : BPC * 128 + D]
    io50 = cs[:, 0:WT]
    ident4 = cs[0:BPC, WT : WT + BPC]

    # zeroed one-hot wv holders + misc (gpsimd is otherwise idle early)
    ghb = {}
    for b in range(BPC):
        ghb[b] = const(f"ghb{b}", [128, BPC], BF16)
        nc.gpsimd.memset(ghb[b][:], 0.0)
    ones_f = const("ones_f", [1, 128])
    nc.gpsimd.memset(ones_f[:], 1.0)

    if DBG_STAGE <= 0:
        out_sb0 = const("out_sb0", [BPC, D])
        nc.vector.tensor_copy(out_sb0[:], pwh[0:BPC, 0, 0:D])
        nc.sync.dma_start(out[:], out_sb0[:])
        return

    # ---- p-chain: P1 = [tgt_hi|tgt_lo]^T @ Wp_hi  (8 rows) ----------------
    psum_p1 = ps.tile([32 + BPC, D], F32, tag="u", name="psum_p1")
    for k in range(KP):
        nc.tensor.matmul(psum_p1[:],
                         lhsT=pwh[:, k, D : D + 32 + BPC],
                         rhs=pwh[:, k, 0:D],
                         start=(k == 0), stop=False)

    with tc.high_priority():
        th = const("th", [BPC, D])
        nc.scalar.activation(th[:], psum_p1[0:BPC, :], AF.Tanh)
        ttr_junk = const("ttr_junk", [BPC, D])
        nc.vector.scalar_tensor_tensor(ttr_junk[:], th[:], 1.0, vp4,
                                       op0=OP.mult, op1=OP.mult,
                                       accum_out=tr_in[0:BPC, 0:1])
        nc.vector.transpose(tr_out[:], tr_in[:])
        # sigmoid(v+bv) = 0.5*tanh(0.5*v + 0.5*bv) + 0.5
        th2_a = const("th2_a", [1, BPC])
        nc.scalar.activation(th2_a[:], tr_out[0:1, 0:BPC], AF.Tanh,
                             bias=bvh_ap, scale=0.5)

        if DBG_STAGE <= 1:
            out_sb1 = const("out_sb1", [BPC, D])
            nc.vector.tensor_copy(out_sb1[:], th[:])
            nc.sync.dma_start(out[:], out_sb1[:])
            return

        # window start: s0 = clamp(trunc(p_approx) - WPOS/2, 0, S - WPOS)
        cf_row = const("cf_row", [1, BPC])
        nc.gpsimd.tensor_scalar(cf_row[:], th2_a[:], float(S) / 2.0,
                                float(S) / 2.0 - WPOS / 2.0,
                                op0=OP.mult, op1=OP.add)
        t0i_row = const("t0i_row", [1, BPC], I32)
        nc.gpsimd.tensor_scalar(t0i_row[:], cf_row[:], 0.0, float(S0_MAX),
                                op0=OP.max, op1=OP.min)
        _, t0v = nc.values_load_multi_w_load_instructions(
            t0i_row[:1, 0:BPC], engines=(ET.SP, ET.Activation),
            min_val=0, max_val=S0_MAX, skip_runtime_bounds_check=True)

        win = {}
        for b in range(BPC):
            win[b] = sb.tile([128, WT, D], F32, tag=f"win{b}", name=f"win{b}")
            eng = nc.sync if b % 2 == 0 else nc.scalar
            eng.dma_start(
                win[b][:],
                src[b][bass.ds(t0v[b], WPOS), :].rearrange("(t p) d -> p t d",
                                                           p=128))

    if DBG_STAGE <= 2:
        out_sb2 = const("out_sb2", [BPC, D])
        for b in range(BPC):
            nc.vector.tensor_copy(out_sb2[b : b + 1, :], win[b][0:1, 0, :])
        nc.sync.dma_start(out[:], out_sb2[:])
        return

    # the lo@hi rows are final after the A-matmuls; stash them in SBUF with
    # bp folded in (off the critical path) so the exact-u add has only one
    # PSUM operand and the p-chain needs no bias matmul
    lo_sb = const("lo_sb", [BPC, D])
    nc.vector.scalar_tensor_tensor(lo_sb[:], psum_p1[32 : 32 + BPC, :], 1.0,
                                   bp4, op0=OP.mult, op1=OP.add)
    # hi@lo accumulates into the hi@hi rows after the approx tanh read
    for k in range(KP):
        nc.tensor.matmul(psum_p1[0:BPC, :],
                         lhsT=pwh[:, k, D : D + BPC],
                         rhs=pwl[:, k, :],
                         start=False, stop=(k == KP - 1))

    # ---- a-chain matmuls --------------------------------------------------
    psum_a = ps.tile([BPC, D], F32, tag="a", name="psum_a")
    for k in range(KP):
        nc.tensor.matmul(psum_a[:],
                         lhsT=pa[:, KP * D + BPC * k : KP * D + BPC * (k + 1)],
                         rhs=pa[:, k * D : (k + 1) * D],
                         start=(k == 0), stop=(k == KP - 1))
    a_bf = const("a_bf", [BPC, D], BF16)
    nc.vector.scalar_tensor_tensor(a_bf[:], psum_a[:], 1.0, ba4,
                                   op0=OP.mult, op1=OP.add)
    ab = ps.tile([128, BPC, D], F32, tag="ab", name="psum_ab")
    for b in range(BPC):
        sel_b = br[:, b * 128 : (b + 1) * 128]
        nc.tensor.matmul(ab[:, b, :], lhsT=sel_b, rhs=a_bf[:],
                         start=True, stop=True)

    # ---- exact u = hi@hi+bp + lo@hi + hi@lo  ->  exact p for g ------------
    if True:
        u_e = const("u_e", [BPC, D])
        nc.vector.scalar_tensor_tensor(u_e[:], psum_p1[0:BPC, :], 1.0,
                                       lo_sb[:], op0=OP.mult, op1=OP.add)
        th_e = const("th_e", [BPC, D])
        nc.scalar.activation(th_e[:], u_e[:], AF.Tanh)
        nc.vector.scalar_tensor_tensor(ttr_junk[:], th_e[:], 1.0, vp4,
                                       op0=OP.mult, op1=OP.mult,
                                       accum_out=tr_in[0:BPC, 0:1])
        nc.vector.transpose(tr_out[:], tr_in[:])
        th2_e = const("th2_e", [1, BPC])
        nc.scalar.activation(th2_e[:], tr_out[0:1, 0:BPC], AF.Tanh,
                             bias=bvh_ap, scale=0.5)

        # g chain: q = t0/50 - p_exact/50, g = exp(-2*(io/50 + q)^2)
        t0f_row = const("t0f_row", [1, BPC])
        nc.gpsimd.tensor_copy(t0f_row[:], t0i_row[:])
        p50_row = const("p50_row", [1, BPC])
        nc.gpsimd.tensor_scalar(p50_row[:], th2_e[:], float(S) / WINDOW / 2.0,
                                float(S) / WINDOW / 2.0,
                                op0=OP.mult, op1=OP.add)
        q_row = const("q_row", [1, BPC])
        nc.gpsimd.tensor_scalar_mul(q_row[:], t0f_row[:], 1.0 / WINDOW)
        nc.gpsimd.tensor_tensor(q_row[:], q_row[:], p50_row[:],
                                op=OP.subtract)
        psum_q = ps.tile([128, BPC], F32, tag="sr", name="psum_q")
        nc.tensor.matmul(psum_q[:], lhsT=ones_f[:], rhs=q_row[:],
                         start=True, stop=True)
        q_bc = const("q_bc", [128, BPC])
        nc.vector.tensor_copy(q_bc[:], psum_q[:])
        ut4 = const("ut4", [128, BPC, WT])
        for b in range(BPC):
            nc.gpsimd.tensor_scalar_add(ut4[:, b, :], io50,
                                        q_bc[:, b : b + 1])
        ut4f = ut4[:].rearrange("p a b -> p (a b)")
        sq4 = const("sq4", [128, BPC * WT])
        nc.gpsimd.tensor_tensor(sq4[:], ut4f, ut4f, op=OP.mult)
        g4 = const("g4", [128, BPC, WT])
        nc.scalar.activation(g4[:].rearrange("p a b -> p (a b)"), sq4[:],
                             AF.Exp, scale=-2.0)

    # ---- per-batch softmax stream -----------------------------------------
    s1c = const("s1c", [128, BPC])
    r1c = const("r1c", [128, BPC])
    s2c = const("s2c", [128, BPC])
    r2c = const("r2c", [128, BPC])
    psum_ctx = psk.tile([BPC, D], F32, tag="ctx", name="psum_ctx")
    for b in range(BPC):
        wb = win[b]
        x = sb.tile([128, WT, D], F32, tag=f"x{b % 2}", name=f"x{b}")
        for t in range(WT):
            nc.vector.tensor_tensor(x[:, t, :], wb[:, t, :], ab[:, b, :],
                                    op=OP.mult)
        e1 = sb.tile([128, WT, D], BF16, tag=f"e1_{b % 2}", name=f"e1_{b}")
        for t in range(WT):
            nc.scalar.activation(e1[:, t, :], x[:, t, :], AF.Exp,
                                 accum_out=s1c[:, b : b + 1])
        nc.vector.reciprocal(r1c[:, b : b + 1], s1c[:, b : b + 1])
        e2 = sb.tile([128, WT, D], BF16, tag=f"e2_{b % 2}", name=f"e2_{b}")
        for t in range(WT):
            nc.scalar.activation(e2[:, t, :], e1[:, t, :], AF.Exp,
                                 scale=r1c[:, b : b + 1],
                                 accum_out=s2c[:, b : b + 1])
        nc.vector.reciprocal(r2c[:, b : b + 1], s2c[:, b : b + 1])
        nc.vector.tensor_tensor(ghb[b][:, b : b + 1], g4[:, b, 0:1],
                                r2c[:, b : b + 1], op=OP.mult)
        t3 = sb.tile([128, WT, D], BF16, tag=f"t3_{b % 2}", name=f"t3_{b}")
        for t in range(WT):
            nc.vector.tensor_tensor(t3[:, t, :], e2[:, t, :], wb[:, t, :],
                                    op=OP.mult)
        for t in range(WT):
            nc.tensor.matmul(psum_ctx[:], lhsT=ghb[b][:], rhs=t3[:, t, :],
                             start=(b == 0 and t == 0),
                             stop=(b == BPC - 1 and t == WT - 1))

    out_sb = const("out_sb", [BPC, D])
    nc.vector.tensor_copy(out_sb[:], psum_ctx[:])
    nc.sync.dma_start(out[:], out_sb[:])


def build_nc():
    nc = bacc.Bacc("TRN2", target_bir_lowering=False, debug=False,
                   num_devices=N_CORES)
    src = nc.dram_tensor("source", [BPC, S, D], F32, kind="ExternalInput").ap()
    packWh = nc.dram_tensor("packWh", [128, KP, PWH_W], BF16,
                            kind="ExternalInput").ap()
    packWl = nc.dram_tensor("packWl", [128, KP, D], BF16,
                            kind="ExternalInput").ap()
    packA = nc.dram_tensor("packA", [128, PA_W], BF16,
                           kind="ExternalInput").ap()
    ptail = nc.dram_tensor("ptail", [BPC, PT_W], F32,
                           kind="ExternalInput").ap()
    brow = nc.dram_tensor("brow", [BPC, BR_W], BF16,
                          kind="ExternalInput").ap()
    cst = nc.dram_tensor("cst", [128, CST_W], F32, kind="ExternalInput").ap()
    out = nc.dram_tensor("out", [BPC, D], F32, kind="ExternalOutput").ap()
    with tile.TileContext(nc) as tc:
        with ExitStack() as ctx:
            _emit(ctx, tc, [out],
                  [src, packWh, packWl, packA, ptail, brow, cst])
    nc.compile()
    return nc


_NC_CACHE = {}


def _get_nc():
    if "nc" not in _NC_CACHE:
        _NC_CACHE["nc"] = build_nc()
    return _NC_CACHE["nc"]


def pack_weights(target_shard, Wp, bp, Wa, ba, Vp, bv):
    """Build the packed weight arrays for one core."""
    import ml_dtypes
    f = np.float32
    wp32 = np.asarray(Wp, f)
    wp_hi = wp32.astype(ml_dtypes.bfloat16)
    wp_lo = (wp32 - wp_hi.astype(f)).astype(ml_dtypes.bfloat16)
    tgt32 = np.asarray(target_shard, f).T                   # [D, BPC]
    tgt_hi = tgt32.astype(ml_dtypes.bfloat16)
    tgt_lo = (tgt32 - tgt_hi.astype(f)).astype(ml_dtypes.bfloat16)
    # per k-chunk: [128, D] Wp_hi rows | [128, BPC] tgt_hi | [128, BPC] tgt_lo
    wph_r = wp_hi.reshape(KP, 128, D)
    tgh_r = tgt_hi.reshape(KP, 128, BPC)
    tgl_r = tgt_lo.reshape(KP, 128, BPC)
    pad_r = np.zeros((KP, 128, 32 - BPC), ml_dtypes.bfloat16)
    packWh = (np.concatenate([wph_r, tgh_r, pad_r, tgl_r], axis=2)
              .transpose(1, 0, 2))                          # [128, KP, D+36]
    packWl = wp_lo.reshape(KP, 128, D).transpose(1, 0, 2)   # [128, KP, D]
    wa_bf = (np.asarray(Wa, f).reshape(KP, 128, D).transpose(1, 0, 2)
             .reshape(128, KP * D).astype(ml_dtypes.bfloat16))
    tgt_bf = (tgt_hi.reshape(KP, 128, BPC).transpose(1, 0, 2)
              .reshape(128, KP * BPC))
    packA = np.concatenate([wa_bf, tgt_bf], axis=1)         # [128, PA_W]
    vp4 = np.broadcast_to(np.asarray(Vp, f).ravel()[None, :], (BPC, D))
    tailc = np.zeros((BPC, 2), f)
    tailc[0, 0] = 0.5 * np.asarray(bv, f).ravel()[0]
    bp4 = np.broadcast_to(np.asarray(bp, f).ravel()[None, :], (BPC, D))
    ptail = np.concatenate([vp4, tailc, bp4], axis=1)       # [BPC, 2D+2]
    brow = np.zeros((BPC, BR_W), ml_dtypes.bfloat16)
    for b in range(BPC):
        brow[b, b * 128 : (b + 1) * 128] = 1.0
    brow[:, BPC * 128 :] = np.broadcast_to(
        np.asarray(ba, f).ravel()[None, :], (BPC, D)).astype(
        ml_dtypes.bfloat16)
    cst = np.zeros((128, CST_W), f)
    io = np.arange(128, dtype=f)[:, None] + 128.0 * np.arange(WT, dtype=f)
    cst[:, 0:WT] = io / np.float32(WINDOW)
    cst[0:BPC, WT : WT + BPC] = np.eye(BPC, dtype=f)
    return (np.ascontiguousarray(packWh), np.ascontiguousarray(packWl),
            np.ascontiguousarray(packA), np.ascontiguousarray(ptail),
            np.ascontiguousarray(brow), np.ascontiguousarray(cst))


def make_in_maps(source, target, Wp, bp, Wa, ba, Vp, bv):
    in_maps = []
    for c in range(N_CORES):
        bs = slice(c * BPC, (c + 1) * BPC)
        packWh, packWl, packA, ptail, brow, cst = pack_weights(
            target[bs], Wp, bp, Wa, ba, Vp, bv)
        in_maps.append({
            "source": np.ascontiguousarray(source[bs], dtype=np.float32),
            "packWh": packWh, "packWl": packWl, "packA": packA,
            "ptail": ptail, "brow": brow, "cst": cst,
        })
    return in_maps


def kernel(source, target, Wp, bp, Wa, ba, Vp, bv, **run_kwargs):
    nc = _get_nc()
    in_maps = make_in_maps(source, target, Wp, bp, Wa, ba, Vp, bv)
    res = run_bass_kernel_spmd(nc, in_maps, core_ids=list(range(N_CORES)),
                               **run_kwargs)
    out = np.concatenate([r["out"] for r in res.results], axis=0)
    kernel.last_results = res
    return out


# revision 34
# speedup vs baseline: 1.0674x; 1.0612x over previous
"""Trainium2 Bass kernel for local (Gaussian-windowed) attention.

Reference computation (per batch b):
    h = target[b]                                # [D]
    p = sigmoid(tanh(h @ Wp + bp) @ Vp + bv) * S # scalar aligned position
    a = h @ Wa + ba                              # [D]
    x[s, d]  = source[b, s, d] * a[d]
    y[s, :]  = softmax(x[s, :])                  # over feature axis
    w[s, :]  = softmax(y[s, :])                  # double softmax
    g[s]     = exp(-2 * ((s - p) / 50)^2)        # Gaussian window
    out[b,d] = sum_s w[s, d] * g[s] * src[b, s, d]

Gaussian window width 50 -> only a 128-position window of `source` around p
matters.  The window offset is computed on-device and used as a
register-dynamic DMA offset.

Key structure:
  - Row-packed p-chain: lhsT = [tgt_hi | tgt_lo] (8 columns) against
    rhs = Wp_hi_k gives hi@hi and lo@hi in ONE matmul (PE cost depends
    only on the free dim), and against Wp_lo_k gives hi@lo.  9 matmuls
    total instead of 13; the row-block sums run on DVE.
  - Two-phase p: tanh of the hi@hi rows (read mid-accumulation) gives an
    approximate p good to ~+-4 positions -- enough to *center* the
    128-position window, so the window DMAs launch ~6us before the exact
    p (needed only for the Gaussian factors g) is finished.
  - DMA rings: FIFO per HWDGE ring gives priority: sync ring carries
    Wp-hi chunks, Wp-lo, Wa; scalar ring the small constant packs, then
    windows alternate rings.
  - Stream per batch: x = win*a (DVE) -> e1 = exp(x) bf16 + accum s1
    (ACT) -> r1 = 1/s1 (DVE) -> e2 = exp(e1*r1) bf16 + accum s2 (ACT
    scale port) -> t3 = e2*win bf16 (DVE) -> matmul with lhsT = one-hot
    column of wv = g/s2 in bf16 (Gaussian factor and second-softmax
    normalisation ride the PE lhsT, off the t3 path).
"""

import os
from contextlib import ExitStack

import numpy as np

import concourse.bass as bass
import concourse.tile as tile
from concourse import bacc, mybir
from concourse.bass_utils import run_bass_kernel_spmd

F32 = mybir.dt.float32
BF16 = mybir.dt.bfloat16
I32 = mybir.dt.int32
AF = mybir.ActivationFunctionType
OP = mybir.AluOpType
ET = mybir.EngineType

N_CORES = 8
B, S, D = 32, 4096, 512
BPC = B // N_CORES          # batches per core
KP = D // 128               # contraction chunks of 128 for D=512
WINDOW = 50.0

WT = 1                      # window tiles; window = 128*WT positions
WPOS = 128 * WT
S0_MAX = S - WPOS
PWH_W = D + 32 + BPC        # packWh cols: Wp_hi | tgt_hi | pad | tgt_lo@32
PA_W = KP * D + KP * BPC    # packA cols: Wa chunks | tgt chunks
PT_W = 2 * D + 2            # ptail cols: vp4 | bvh | pad | bp4
CST_W = WT + BPC            # cst cols: io50 | ident4
BR_W = BPC * 128 + D        # brow cols: sel rows | ba4

DBG_STAGE = int(os.environ.get("DBG_STAGE", "4"))


def _emit(ctx: ExitStack, tc: tile.TileContext, outs, ins):
    nc = tc.nc
    (out,) = outs
    (src, packWh, packWl, packA, ptail, brow, cst) = ins

    sb = ctx.enter_context(tc.tile_pool(name="sb", bufs=1))
    ps = ctx.enter_context(tc.tile_pool(name="ps", bufs=1, space="PSUM"))
    psk = ctx.enter_context(tc.tile_pool(name="psk", bufs=1, space="PSUM"))

    def const(name, shape, dtype=F32):
        return sb.tile(shape, dtype, tag=name, name=name)

    # ---- memset constants --------------------------------------------------
    tr_in = const("tr_in", [32, 32])
    nc.gpsimd.memset(tr_in[:], 0.0)
    tr_out = const("tr_out", [32, 32])

    # ---- weight DMAs (ring FIFO gives priority ordering) ------------------
    # sync ring: Wp-hi chunk A, Wp-hi chunk B, Wp-lo, Wa pack
    # scalar ring: small constant packs (ptail, cst, brow)
    pwh = const("pwh", [128, KP, PWH_W], BF16)
    nc.sync.dma_start(pwh[:, 0:2, :], packWh[:, 0:2, :])
    nc.sync.dma_start(pwh[:, 2:4, :], packWh[:, 2:4, :])
    pwl = const("pwl", [128, KP, D], BF16)
    nc.sync.dma_start(pwl[:], packWl[:])
    pa = const("pa", [128, PA_W], BF16)
    nc.sync.dma_start(pa[:], packA[:])
    pt = const("pt", [BPC, PT_W])
    nc.scalar.dma_start(pt[:], ptail[:])
    cs = const("cs", [128, CST_W])
    nc.scalar.dma_start(cs[:], cst[:])
    br = const("br", [BPC, BR_W], BF16)
    nc.scalar.dma_start(br[:], brow[:])

    vp4 = pt[:, 0:D]
    bvh_ap = pt[0:1, D : D + 1]
    bp4 = pt[:, D + 2 : 2 * D + 2]
    ba4 = br[:, BPC * 128 : BPC * 128 + D]
    io50 = cs[:, 0:WT]
    ident4 = cs[0:BPC, WT : WT + BPC]

    # zeroed one-hot wv holders + misc (gpsimd is otherwise idle early)
    ghb = {}
    for b in range(BPC):
        ghb[b] = const(f"ghb{b}", [128, BPC], BF16)
        nc.gpsimd.memset(ghb[b][:], 0.0)
    ones_f = const("ones_f", [1, 128])
    nc.gpsimd.memset(ones_f[:], 1.0)

    if DBG_STAGE <= 0:
        out_sb0 = const("out_sb0", [BPC, D])
        nc.vector.tensor_copy(out_sb0[:], pwh[0:BPC, 0, 0:D])
        nc.sync.dma_start(out[:], out_sb0[:])
        return

    # ---- p-chain: P1 = [tgt_hi|tgt_lo]^T @ Wp_hi  (8 rows) ----------------
    psum_p1 = ps.tile([32 + BPC, D], F32, tag="u", name="psum_p1")
    for k in range(KP):
        nc.tensor.matmul(psum_p1[:],
                         lhsT=pwh[:, k, D : D + 32 + BPC],
                         rhs=pwh[:, k, 0:D],
                         start=(k == 0), stop=(k == KP - 1))

    with tc.high_priority():
        th = const("th", [BPC, D])
        nc.scalar.activation(th[:], psum_p1[0:BPC, :], AF.Tanh)
        ttr_junk = const("ttr_junk", [BPC, D])
        nc.vector.scalar_tensor_tensor(ttr_junk[:], th[:], 1.0, vp4,
                                       op0=OP.mult, op1=OP.mult,
                                       accum_out=tr_in[0:BPC, 0:1])
        nc.vector.transpose(tr_out[:], tr_in[:])
        # sigmoid(v+bv) = 0.5*tanh(0.5*v + 0.5*bv) + 0.5
        th2_a = const("th2_a", [1, BPC])
        nc.scalar.activation(th2_a[:], tr_out[0:1, 0:BPC], AF.Tanh,
                             bias=bvh_ap, scale=0.5)

        if DBG_STAGE <= 1:
            out_sb1 = const("out_sb1", [BPC, D])
            nc.vector.tensor_copy(out_sb1[:], th[:])
            nc.sync.dma_start(out[:], out_sb1[:])
            return

        # window start: s0 = clamp(trunc(p_approx) - WPOS/2, 0, S - WPOS)
        cf_row = const("cf_row", [1, BPC])
        nc.gpsimd.tensor_scalar(cf_row[:], th2_a[:], float(S) / 2.0,
                                float(S) / 2.0 - WPOS / 2.0,
                                op0=OP.mult, op1=OP.add)
        t0i_row = const("t0i_row", [1, BPC], I32)
        nc.gpsimd.tensor_scalar(t0i_row[:], cf_row[:], 0.0, float(S0_MAX),
                                op0=OP.max, op1=OP.min)
        _, t0v = nc.values_load_multi_w_load_instructions(
            t0i_row[:1, 0:BPC], engines=(ET.SP, ET.Activation),
            min_val=0, max_val=S0_MAX, skip_runtime_bounds_check=True)

        win = {}
        for b in range(BPC):
            win[b] = sb.tile([128, WT, D], F32, tag=f"win{b}", name=f"win{b}")
            eng = nc.sync if b % 2 == 0 else nc.scalar
            eng.dma_start(
                win[b][:],
                src[b][bass.ds(t0v[b], WPOS), :].rearrange("(t p) d -> p t d",
                                                           p=128))

    if DBG_STAGE <= 2:
        out_sb2 = const("out_sb2", [BPC, D])
        for b in range(BPC):
            nc.vector.tensor_copy(out_sb2[b : b + 1, :], win[b][0:1, 0, :])
        nc.sync.dma_start(out[:], out_sb2[:])
        return

    # hi@lo in its own PSUM bank -- no dependence on the approx-tanh read,
    # so these run as soon as Wp_lo arrives
    psum_pB = ps.tile([BPC, D], F32, tag="sr", name="psum_pB")
    for k in range(KP):
        nc.tensor.matmul(psum_pB[:],
                         lhsT=pwh[:, k, D : D + BPC],
                         rhs=pwl[:, k, :],
                         start=(k == 0), stop=(k == KP - 1))
    # hi@hi rows with bp folded in (single-PSUM-operand adds)
    hi_sb = const("hi_sb", [BPC, D])
    nc.vector.scalar_tensor_tensor(hi_sb[:], psum_p1[0:BPC, :], 1.0,
                                   bp4, op0=OP.mult, op1=OP.add)
    u_mid = const("u_mid", [BPC, D])
    nc.vector.scalar_tensor_tensor(u_mid[:], psum_pB[:], 1.0,
                                   hi_sb[:], op0=OP.mult, op1=OP.add)

    # ---- a-chain matmuls --------------------------------------------------
    psum_a = ps.tile([BPC, D], F32, tag="a", name="psum_a")
    for k in range(KP):
        nc.tensor.matmul(psum_a[:],
                         lhsT=pa[:, KP * D + BPC * k : KP * D + BPC * (k + 1)],
                         rhs=pa[:, k * D : (k + 1) * D],
                         start=(k == 0), stop=(k == KP - 1))
    a_bf = const("a_bf", [BPC, D], BF16)
    nc.vector.scalar_tensor_tensor(a_bf[:], psum_a[:], 1.0, ba4,
                                   op0=OP.mult, op1=OP.add)
    ab = ps.tile([128, BPC, D], F32, tag="ab", name="psum_ab")
    for b in range(BPC):
        sel_b = br[:, b * 128 : (b + 1) * 128]
        nc.tensor.matmul(ab[:, b, :], lhsT=sel_b, rhs=a_bf[:],
                         start=True, stop=True)

    # ---- exact u = hi@hi+bp + lo@hi + hi@lo  ->  exact p for g ------------
    if True:
        u_e = const("u_e", [BPC, D])
        nc.vector.scalar_tensor_tensor(u_e[:], psum_p1[32 : 32 + BPC, :], 1.0,
                                       u_mid[:], op0=OP.mult, op1=OP.add)
        th_e = const("th_e", [BPC, D])
        nc.scalar.activation(th_e[:], u_e[:], AF.Tanh)
        nc.vector.scalar_tensor_tensor(ttr_junk[:], th_e[:], 1.0, vp4,
                                       op0=OP.mult, op1=OP.mult,
                                       accum_out=tr_in[0:BPC, 0:1])
        nc.vector.transpose(tr_out[:], tr_in[:])
        th2_e = const("th2_e", [1, BPC])
        nc.scalar.activation(th2_e[:], tr_out[0:1, 0:BPC], AF.Tanh,
                             bias=bvh_ap, scale=0.5)

        # g chain: q = t0/50 - p_exact/50, g = exp(-2*(io/50 + q)^2)
        t0f_row = const("t0f_row", [1, BPC])
        nc.gpsimd.tensor_copy(t0f_row[:], t0i_row[:])
        p50_row = const("p50_row", [1, BPC])
        nc.gpsimd.tensor_scalar(p50_row[:], th2_e[:], float(S) / WINDOW / 2.0,
                                float(S) / WINDOW / 2.0,
                                op0=OP.mult, op1=OP.add)
        q_row = const("q_row", [1, BPC])
        nc.gpsimd.tensor_scalar_mul(q_row[:], t0f_row[:], 1.0 / WINDOW)
        nc.gpsimd.tensor_tensor(q_row[:], q_row[:], p50_row[:],
                                op=OP.subtract)
        psum_q = ps.tile([128, BPC], F32, tag="sr", name="psum_q")
        nc.tensor.matmul(psum_q[:], lhsT=ones_f[:], rhs=q_row[:],
                         start=True, stop=True)
        ut4 = const("ut4", [128, BPC, WT])
        for b in range(BPC):
            nc.vector.scalar_tensor_tensor(ut4[:, b, :], psum_q[:, b : b + 1],
                                           1.0, io50,
                                           op0=OP.mult, op1=OP.add)
        ut4f = ut4[:].rearrange("p a b -> p (a b)")
        sq4 = const("sq4", [128, BPC * WT])
        nc.vector.tensor_tensor(sq4[:], ut4f, ut4f, op=OP.mult)
        g4 = const("g4", [128, BPC, WT])
        nc.scalar.activation(g4[:].rearrange("p a b -> p (a b)"), sq4[:],
                             AF.Exp, scale=-2.0)

    # ---- per-batch softmax stream -----------------------------------------
    s1c = const("s1c", [128, BPC])
    r1c = const("r1c", [128, BPC])
    s2c = const("s2c", [128, BPC])
    r2c = const("r2c", [128, BPC])
    psum_ctx = psk.tile([BPC, D], F32, tag="ctx", name="psum_ctx")
    for b in range(BPC):
        wb = win[b]
        x = sb.tile([128, WT, D], F32, tag=f"x{b % 2}", name=f"x{b}")
        for t in range(WT):
            nc.vector.tensor_tensor(x[:, t, :], wb[:, t, :], ab[:, b, :],
                                    op=OP.mult)
        e1 = sb.tile([128, WT, D], BF16, tag=f"e1_{b % 2}", name=f"e1_{b}")
        for t in range(WT):
            nc.scalar.activation(e1[:, t, :], x[:, t, :], AF.Exp,
                                 accum_out=s1c[:, b : b + 1])
        nc.vector.reciprocal(r1c[:, b : b + 1], s1c[:, b : b + 1])
        e2 = sb.tile([128, WT, D], BF16, tag=f"e2_{b % 2}", name=f"e2_{b}")
        for t in range(WT):
            nc.scalar.activation(e2[:, t, :], e1[:, t, :], AF.Exp,
                                 scale=r1c[:, b : b + 1],
                                 accum_out=s2c[:, b : b + 1])
        nc.vector.reciprocal(r2c[:, b : b + 1], s2c[:, b : b + 1])
        nc.vector.tensor_tensor(ghb[b][:, b : b + 1], g4[:, b, 0:1],
                                r2c[:, b : b + 1], op=OP.mult)
        t3 = sb.tile([128, WT, D], BF16, tag=f"t3_{b % 2}", name=f"t3_{b}")
        for t in range(WT):
            nc.vector.tensor_tensor(t3[:, t, :], e2[:, t, :], wb[:, t, :],
                                    op=OP.mult)
        for t in range(WT):
            nc.tensor.matmul(psum_ctx[:], lhsT=ghb[b][:], rhs=t3[:, t, :],
                             start=(b == 0 and t == 0),
                             stop=(b == BPC - 1 and t == WT - 1))

    out_sb = const("out_sb", [BPC, D])
    nc.vector.tensor_copy(out_sb[:], psum_ctx[:])
    nc.sync.dma_start(out[:], out_sb[:])


def build_nc():
    nc = bacc.Bacc("TRN2", target_bir_lowering=False, debug=False,
                   num_devices=N_CORES)
    src = nc.dram_tensor("source", [BPC, S, D], F32, kind="ExternalInput").ap()
    packWh = nc.dram_tensor("packWh", [128, KP, PWH_W], BF16,
                            kind="ExternalInput").ap()
    packWl = nc.dram_tensor("packWl", [128, KP, D], BF16,
                            kind="ExternalInput").ap()
    packA = nc.dram_tensor("packA", [128, PA_W], BF16,
                           kind="ExternalInput").ap()
    ptail = nc.dram_tensor("ptail", [BPC, PT_W], F32,
                           kind="ExternalInput").ap()
    brow = nc.dram_tensor("brow", [BPC, BR_W], BF16,
                          kind="ExternalInput").ap()
    cst = nc.dram_tensor("cst", [128, CST_W], F32, kind="ExternalInput").ap()
    out = nc.dram_tensor("out", [BPC, D], F32, kind="ExternalOutput").ap()
    with tile.TileContext(nc) as tc:
        with ExitStack() as ctx:
            _emit(ctx, tc, [out],
                  [src, packWh, packWl, packA, ptail, brow, cst])
    nc.compile()
    return nc


_NC_CACHE = {}


def _get_nc():
    if "nc" not in _NC_CACHE:
        _NC_CACHE["nc"] = build_nc()
    return _NC_CACHE["nc"]


def pack_weights(target_shard, Wp, bp, Wa, ba, Vp, bv):
    """Build the packed weight arrays for one core."""
    import ml_dtypes
    f = np.float32
    wp32 = np.asarray(Wp, f)
    wp_hi = wp32.astype(ml_dtypes.bfloat16)
    wp_lo = (wp32 - wp_hi.astype(f)).astype(ml_dtypes.bfloat16)
    tgt32 = np.asarray(target_shard, f).T                   # [D, BPC]
    tgt_hi = tgt32.astype(ml_dtypes.bfloat16)
    tgt_lo = (tgt32 - tgt_hi.astype(f)).astype(ml_dtypes.bfloat16)
    # per k-chunk: [128, D] Wp_hi rows | [128, BPC] tgt_hi | [128, BPC] tgt_lo
    wph_r = wp_hi.reshape(KP, 128, D)
    tgh_r = tgt_hi.reshape(KP, 128, BPC)
    tgl_r = tgt_lo.reshape(KP, 128, BPC)
    pad_r = np.zeros((KP, 128, 32 - BPC), ml_dtypes.bfloat16)
    packWh = (np.concatenate([wph_r, tgh_r, pad_r, tgl_r], axis=2)
              .transpose(1, 0, 2))                          # [128, KP, D+36]
    packWl = wp_lo.reshape(KP, 128, D).transpose(1, 0, 2)   # [128, KP, D]
    wa_bf = (np.asarray(Wa, f).reshape(KP, 128, D).transpose(1, 0, 2)
             .reshape(128, KP * D).astype(ml_dtypes.bfloat16))
    tgt_bf = (tgt_hi.reshape(KP, 128, BPC).transpose(1, 0, 2)
              .reshape(128, KP * BPC))
    packA = np.concatenate([wa_bf, tgt_bf], axis=1)         # [128, PA_W]
    vp4 = np.broadcast_to(np.asarray(Vp, f).ravel()[None, :], (BPC, D))
    tailc = np.zeros((BPC, 2), f)
    tailc[0, 0] = 0.5 * np.asarray(bv, f).ravel()[0]
    bp4 = np.broadcast_to(np.asarray(bp, f).ravel()[None, :], (BPC, D))
    ptail = np.concatenate([vp4, tailc, bp4], axis=1)       # [BPC, 2D+2]
    brow = np.zeros((BPC, BR_W), ml_dtypes.bfloat16)
    for b in range(BPC):
        brow[b, b * 128 : (b + 1) * 128] = 1.0
    brow[:, BPC * 128 :] = np.broadcast_to(
        np.asarray(ba, f).ravel()[None, :], (BPC, D)).astype(
        ml_dtypes.bfloat16)
    cst = np.zeros((128, CST_W), f)
    io = np.arange(128, dtype=f)[:, None] + 128.0 * np.arange(WT, dtype=f)
    cst[:, 0:WT] = io / np.float32(WINDOW)
    cst[0:BPC, WT : WT + BPC] = np.eye(BPC, dtype=f)
    return (np.ascontiguousarray(packWh), np.ascontiguousarray(packWl),
            np.ascontiguousarray(packA), np.ascontiguousarray(ptail),
            np.ascontiguousarray(brow), np.ascontiguousarray(cst))


def make_in_maps(source, target, Wp, bp, Wa, ba, Vp, bv):
    in_maps = []
    for c in range(N_CORES):
        bs = slice(c * BPC, (c + 1) * BPC)
        packWh, packWl, packA, ptail, brow, cst = pack_weights(
            target[bs], Wp, bp, Wa, ba, Vp, bv)
        in_maps.append({
            "source": np.ascontiguousarray(source[bs], dtype=np.float32),
            "packWh": packWh, "packWl": packWl, "packA": packA,
            "ptail": ptail, "brow": brow, "cst": cst,
        })
    return in_maps


def kernel(source, target, Wp, bp, Wa, ba, Vp, bv, **run_kwargs):
    nc = _get_nc()
    in_maps = make_in_maps(source, target, Wp, bp, Wa, ba, Vp, bv)
    res = run_bass_kernel_spmd(nc, in_maps, core_ids=list(range(N_CORES)),
                               **run_kwargs)
    out = np.concatenate([r["out"] for r in res.results], axis=0)
    kernel.last_results = res
    return out
